# revision 7
# baseline (speedup 1.0000x reference)
"""BiLSTM-CRF forward (log partition) on 8 trn2 NeuronCores — single launch.

Architecture (per call, after one-time cached setup):
  host:  tokens [512,64] -> per-core token blocks (128KB shipped).
  jit1 (XLA, cached):  xp = WE[tokens] gather + transpose, on device.
         WE[v] = concat_d(w_ih_d @ embed[v] + b_d)  (precomputed on device
         at setup; folds the whole input-projection GEMM + embedding gather
         into one table lookup).
  jit2 (Bass, cached): per core = 8 batch columns end-to-end:
         fwd+bwd LSTM scans (interleaved per step), encoder GEMM + tanh,
         emission GEMM + exp, CRF forward scan in exp domain with periodic
         rescaling -> zbuf/afin (tiny outputs).
  host:  log_z = sum(log zbuf) + log(etstop @ afin).

All weights and the WE table live on device between calls; both jits are
compiled once and cached in-process. Per-call traffic: ~128KB in, ~20KB out.
"""
import numpy as np
import ml_dtypes

import concourse.bass as bass
import concourse.mybir as mybir
import concourse.tile as tile

T, B, E, H, V, K = 512, 64, 256, 512, 50000, 50
P = 128
CB = 8             # batch columns per core
NG = 16            # gate tiles (4H/128)
NK = 4             # h chunks (H/128)
GRP = 32           # scan steps per xp prefetch group
NGRP = T // GRP    # 16
TB2 = T * CB       # 4096
NBLK2 = TB2 // 512 # 8
NZ = T // 16       # 32 rescale slots
AF = mybir.ActivationFunctionType
BF16 = mybir.dt.bfloat16
F32 = mybir.dt.float32
BF = ml_dtypes.bfloat16

GPERM = np.concatenate([
    np.arange(0, 512), np.arange(512, 1024),
    np.arange(1536, 2048), np.arange(1024, 1536)])  # i,f,o,g tile order

_C = {}


def _fix_sync_waits(nc, max_waits=1):
    import bass_rust
    for fn in nc.m.functions:
        for bb in fn.blocks:
            out = []
            for inst in bb.instructions:
                si = inst.sync_info
                if si is not None and si.on_wait and len(si.on_wait) > max_waits:
                    waits = list(si.on_wait)
                    extra, keep = waits[:-max_waits], waits[-max_waits:]
                    for j in range(0, len(extra), max_waits):
                        nop = mybir.InstNoOp(name=f"{inst.name}_ws{j}", ins=[], outs=[])
                        nop.engine = inst.engine
                        nop.sync_info = bass_rust.SyncInfo(
                            on_wait=extra[j:j + max_waits], on_update=[])
                        out.append(nop)
                    inst.sync_info = bass_rust.SyncInfo(
                        on_wait=keep, on_update=list(si.on_update or []))
                out.append(inst)
            bb.instructions = out


def build_fused():
    nc = bass.Bass()
    dp = nc.declare_dram_parameter
    xp_in = dp("xp", [2, NG, P, TB2], BF16, isOutput=False)
    whh_in = dp("whh", [P, 2 * NG * NK, P], BF16, isOutput=False)
    wenc_in = dp("wenc", [P, 2 * NK * NK, P], BF16, isOutput=False)
    benc_in = dp("benc", [P, NK], F32, isOutput=False)
    wout_in = dp("wout", [P, NK, K], BF16, isOutput=False)
    bout_in = dp("bout", [K, 1], F32, isOutput=False)
    pp_in = dp("pp", [K, K + 2], BF16, isOutput=False)
    a0_in = dp("a0", [K, CB], BF16, isOutput=False)
    zbuf_out = dp("zbuf", [1, CB, NZ], F32, isOutput=True)
    afin_out = dp("afin", [K, CB], F32, isOutput=True)

    with tile.TileContext(nc) as tc:
        with tc.tile_pool(name="hseq", bufs=1) as hp:
            hs = [hp.tile([P, NK, TB2], BF16, name=f"hseq{d}", tag=f"hseq{d}")
                  for d in range(2)]

            # ---------------- LSTM scans (fwd+bwd interleaved) -----------
            with (
                tc.tile_pool(name="scanw", bufs=1) as sw,
                tc.tile_pool(name="ps", bufs=2, space="PSUM") as psp,
            ):
                whh = sw.tile([P, 2 * NG * NK, P], BF16)
                nc.sync.dma_start(whh[:], whh_in[:])
                h0 = sw.tile([P, NK * CB], BF16)
                nc.any.memset(h0[:], 0.0)
                xr = [sw.tile([P, 2, NG, GRP, CB], BF16, name=f"xr{d}", tag=f"xr{d}")
                      for d in range(2)]
                cst, gs, sio, tg, m1, m2, tcc = [], [], [], [], [], [], []
                for d in range(2):
                    cst.append(sw.tile([P, NK * CB], F32, name=f"cst{d}", tag=f"cst{d}"))
                    nc.any.memset(cst[d][:], 0.0)
                    gs.append(sw.tile([P, NG * CB], F32, name=f"gs{d}", tag=f"gs{d}"))
                    sio.append(sw.tile([P, 3 * NK * CB], F32, name=f"sio{d}", tag=f"sio{d}"))
                    tg.append(sw.tile([P, NK * CB], F32, name=f"tg{d}", tag=f"tg{d}"))
                    m1.append(sw.tile([P, NK * CB], F32, name=f"m1{d}", tag=f"m1{d}"))
                    m2.append(sw.tile([P, NK * CB], F32, name=f"m2{d}", tag=f"m2{d}"))
                    tcc.append(sw.tile([P, NK * CB], F32, name=f"tcc{d}", tag=f"tcc{d}"))

                def prefetch(g, d):
                    if g >= NGRP:
                        return
                    blk = g if d == 0 else NGRP - 1 - g
                    for m in range(NG):
                        nc.sync.dma_start(
                            xr[d][:, g % 2, m, :, :].rearrange("p t b -> p (t b)"),
                            xp_in[d, m, :, blk * GRP * CB:(blk + 1) * GRP * CB])

                for g in range(2):
                    for d in range(2):
                        prefetch(g, d)

                for g in range(NGRP):
                    for tm in range(GRP):
                        s = g * GRP + tm
                        for d in range(2):
                            t = s if d == 0 else T - 1 - s
                            tl = tm if d == 0 else GRP - 1 - tm
                            if s == 0:
                                hin = h0[:].rearrange("p (a b) -> p a b", b=CB)
                            else:
                                tp = t - 1 if d == 0 else t + 1
                                hin = hs[d][:, :, tp * CB:(tp + 1) * CB]
                            gp = psp.tile([P, NG * CB], F32, tag=f"gp{d}")
                            for m in range(NG):
                                for k in range(NK):
                                    nc.tensor.matmul(
                                        gp[:, m * CB:(m + 1) * CB],
                                        lhsT=whh[:, d * NG * NK + m * NK + k, :],
                                        rhs=hin[:, k, :],
                                        start=(k == 0), stop=(k == NK - 1))
                            nc.vector.tensor_tensor(
                                gs[d][:].rearrange("p (m b) -> p m b", b=CB),
                                gp[:].rearrange("p (m b) -> p m b", b=CB),
                                xr[d][:, g % 2, :, tl, :], mybir.AluOpType.add)
                            nc.scalar.activation(sio[d][:], gs[d][:, 0:3 * NK * CB],
                                                 AF.Sigmoid)
                            nc.scalar.activation(tg[d][:],
                                                 gs[d][:, 3 * NK * CB:4 * NK * CB],
                                                 AF.Tanh)
                            nc.vector.tensor_mul(m1[d][:], sio[d][:, 0:NK * CB],
                                                 tg[d][:])
                            nc.vector.tensor_mul(m2[d][:],
                                                 sio[d][:, NK * CB:2 * NK * CB],
                                                 cst[d][:])
                            nc.vector.tensor_add(cst[d][:], m1[d][:], m2[d][:])
                            nc.scalar.activation(tcc[d][:], cst[d][:], AF.Tanh)
                            nc.vector.tensor_mul(
                                hs[d][:, :, t * CB:(t + 1) * CB],
                                sio[d][:, 2 * NK * CB:3 * NK * CB].rearrange(
                                    "p (a b) -> p a b", b=CB),
                                tcc[d][:].rearrange("p (a b) -> p a b", b=CB))
                    for d in range(2):
                        prefetch(g + 2, d)

            # ---------------- encoder + emissions + CRF ------------------
            with (
                tc.tile_pool(name="enc", bufs=1) as ec,
                tc.tile_pool(name="eps", bufs=2, space="PSUM") as eps,
            ):
                wenc = ec.tile([P, 2 * NK * NK, P], BF16)
                nc.sync.dma_start(wenc[:], wenc_in[:])
                benc = ec.tile([P, NK], F32)
                nc.sync.dma_start(benc[:], benc_in[:])
                wout = ec.tile([P, NK, K], BF16)
                nc.sync.dma_start(wout[:], wout_in[:])
                bout = ec.tile([K, 1], F32)
                nc.sync.dma_start(bout[:], bout_in[:])
                states = ec.tile([P, NK, TB2], BF16)

                for blk in range(NBLK2):
                    sl = slice(blk * 512, (blk + 1) * 512)
                    for m in range(NK):
                        ps = eps.tile([P, 512], F32, tag="enc")
                        for k in range(NK):
                            nc.tensor.matmul(ps[:], lhsT=wenc[:, m * NK + k, :],
                                             rhs=hs[0][:, k, sl],
                                             start=(k == 0), stop=False)
                        for k in range(NK):
                            nc.tensor.matmul(ps[:],
                                             lhsT=wenc[:, NK * NK + m * NK + k, :],
                                             rhs=hs[1][:, k, sl], start=False,
                                             stop=(k == NK - 1))
                        nc.scalar.activation(states[:, m, sl], ps[:], AF.Tanh,
                                             bias=benc[:, m:m + 1])

                expE = ec.tile([K, TB2], F32)
                for blk in range(NBLK2):
                    sl = slice(blk * 512, (blk + 1) * 512)
                    ps = eps.tile([K, 512], F32, tag="emit")
                    for k in range(NK):
                        nc.tensor.matmul(ps[:], lhsT=wout[:, k, :],
                                         rhs=states[:, k, sl],
                                         start=(k == 0), stop=(k == NK - 1))
                    nc.scalar.activation(expE[:, sl], ps[:], AF.Exp,
                                         bias=bout[:, 0:1])

                pp = ec.tile([K, K + 2], BF16)
                nc.sync.dma_start(pp[:], pp_in[:])
                ones_r = ec.tile([1, K], BF16)
                nc.any.memset(ones_r[:], 1.0)
                A = ec.tile([K, CB], BF16)
                nc.sync.dma_start(A[:], a0_in[:])
                zbuf = ec.tile([1, CB, NZ], F32)
                izb = ec.tile([1, CB], F32)
                izb_bf = ec.tile([1, CB], BF16)

                for t in range(T):
                    ps = eps.tile([K, CB], F32, tag="crf", bufs=1)
                    nc.tensor.matmul(ps[:], lhsT=pp[:, 0:K], rhs=A[:],
                                     start=True, stop=True)
                    if t % 16 == 15:
                        r = t // 16
                        zps = eps.tile([1, CB], F32, tag="zps", bufs=1)
                        nc.tensor.matmul(zps[:], lhsT=pp[:, K:K + 1], rhs=A[:],
                                         start=True, stop=True)
                        nc.vector.tensor_copy(zbuf[:, :, r], zps[:])
                        nc.vector.reciprocal(izb[:], zps[:])
                        nc.vector.tensor_copy(izb_bf[:], izb[:])
                        zb = eps.tile([K, CB], F32, tag="zbc", bufs=1)
                        nc.tensor.matmul(zb[:], lhsT=ones_r[:], rhs=izb_bf[:],
                                         start=True, stop=True)
                        nc.vector.tensor_mul(A[:], ps[:],
                                             expE[:, t * CB:(t + 1) * CB])
                        nc.vector.tensor_mul(A[:], A[:], zb[:])
                    else:
                        nc.vector.tensor_mul(A[:], ps[:],
                                             expE[:, t * CB:(t + 1) * CB])

                nc.sync.dma_start(zbuf_out[:], zbuf[:])
                af = ec.tile([K, CB], F32)
                nc.vector.tensor_copy(af[:], A[:])
                nc.sync.dma_start(afin_out[:], af[:])

    _fix_sync_waits(nc)
    return nc


def _tiles_T(W, nm, nk):
    """W [nm*128, nk*128] -> [128, nm*nk, 128] with [:, m*nk+k, :] = block(m,k).T"""
    return np.ascontiguousarray(
        W.reshape(nm, P, nk, P).transpose(3, 0, 2, 1).reshape(P, nm * nk, P))


def _rep8(a):
    """concat 8 copies along axis 0 (replicated shard_map input)."""
    return np.ascontiguousarray(
        np.broadcast_to(a, (8,) + a.shape).reshape((8 * a.shape[0],) + a.shape[1:]))


def _setup(embed, w_ih_f, b_f, w_ih_b, b_b, w_hh_f, w_hh_b,
           w_enc, b_enc, w_out, b_out, trans):
    import jax
    import jax.numpy as jnp
    from jax.experimental.shard_map import shard_map
    from jax.sharding import Mesh, PartitionSpec as PS, NamedSharding
    from concourse.bass2jax import (install_neuronx_cc_hook, _bass_exec_p,
                                    partition_id_tensor)

    install_neuronx_cc_hook()
    devs = jax.devices()[:8]
    mesh = Mesh(np.asarray(devs), ("core",))
    shard = NamedSharding(mesh, PS("core"))
    _C["mesh"] = mesh
    _C["shard"] = shard
    _C["jax"] = jax

    # ---- WE table: WE[v, d, m, p] = (w_ih_d[GPERM] @ embed[v]) + b_d[GPERM]
    wihg = np.stack([w_ih_f[GPERM], w_ih_b[GPERM]]).astype(BF)      # [2,4H,E]
    bg = np.stack([b_f[GPERM], b_b[GPERM]]).astype(np.float32)      # [2,4H]
    embed_b = embed.astype(BF)                                      # [V,E]

    def we_fn(embed_c, wihg_c, bg_c):
        out = jnp.einsum("ve,dge->vdg", embed_c, wihg_c,
                         preferred_element_type=jnp.float32)
        out = out + bg_c[None]
        return out.astype(jnp.bfloat16).reshape(V, 2, NG, P)

    we_jit = jax.jit(shard_map(we_fn, mesh=mesh, in_specs=(PS("core"),) * 3,
                               out_specs=PS("core")))
    emb_dev = jax.device_put(_rep8(embed_b), shard)
    wihg_dev = jax.device_put(_rep8(wihg), shard)
    bg_dev = jax.device_put(_rep8(bg), shard)
    WE_dev = we_jit(emb_dev, wihg_dev, bg_dev)
    WE_dev.block_until_ready()
    del emb_dev, wihg_dev, bg_dev
    _C["WE"] = WE_dev

    # ---- gather jit: tokens -> xp layout [2, NG, P, TB2] per core
    def gather_fn(tok_c, WE_c):
        xp = WE_c[tok_c.reshape(-1)]          # [TB2, 2, NG, P]
        return jnp.transpose(xp, (1, 2, 3, 0))

    _C["gather"] = jax.jit(shard_map(
        gather_fn, mesh=mesh, in_specs=(PS("core"), PS("core")),
        out_specs=PS("core")))

    # ---- device-resident bass weights
    whh_t = np.concatenate(
        [_tiles_T(w_hh_f[GPERM], NG, NK), _tiles_T(w_hh_b[GPERM], NG, NK)],
        axis=1).astype(BF)                                          # [P,128,P]
    wenc_t = np.concatenate(
        [_tiles_T(w_enc[:, :H], NK, NK), _tiles_T(w_enc[:, H:], NK, NK)],
        axis=1).astype(BF)                                          # [P,32,P]
    benc_t = np.ascontiguousarray(b_enc.reshape(NK, P).T).astype(np.float32)
    wout_t = np.ascontiguousarray(
        w_out.reshape(K, NK, P).transpose(2, 1, 0)).astype(BF)      # [P,NK,K]
    bout_t = b_out.reshape(K, 1).astype(np.float32)
    ppm = np.zeros((K, K + 2), np.float32)
    ppm[:, :K] = np.exp(trans.astype(np.float64)).T.astype(np.float32)
    ppm[:, K] = 1.0
    ppm[:, K + 1] = np.exp(trans[K - 1].astype(np.float64)).astype(np.float32)
    a0 = np.zeros((K, CB), np.float32)
    a0[0, :] = 1.0

    wdevs = {}
    for name, arr in [("whh", whh_t), ("wenc", wenc_t), ("benc", benc_t),
                      ("wout", wout_t), ("bout", bout_t),
                      ("pp", ppm.astype(BF)), ("a0", a0.astype(BF))]:
        wdevs[name] = jax.device_put(_rep8(arr), shard)
    _C["wdevs"] = wdevs
    _C["etstop"] = np.exp(trans[K - 1].astype(np.float64))

    # ---- bass jit (built once, cached)
    nc = build_fused()
    part_name = nc.partition_id_tensor.name if nc.partition_id_tensor else None
    in_names, out_names, out_avals, zero_shapes = [], [], [], []
    for alloc in nc.m.functions[0].allocations:
        if not isinstance(alloc, mybir.MemoryLocationSet):
            continue
        name = alloc.memorylocations[0].name
        if alloc.kind == "ExternalInput":
            if name != part_name:
                in_names.append(name)
        elif alloc.kind == "ExternalOutput":
            out_names.append(name)
            shape = tuple(alloc.tensor_shape)
            dtype = mybir.dt.np(alloc.dtype)
            out_avals.append(jax.core.ShapedArray(shape, dtype))
            zero_shapes.append((shape, dtype))
    assert in_names == ["xp", "whh", "wenc", "benc", "wout", "bout", "pp", "a0"], in_names
    assert out_names == ["zbuf", "afin"], out_names
    n_params = len(in_names)
    all_names = in_names + out_names
    if part_name is not None:
        all_names = all_names + [part_name]
    donate = tuple(range(n_params, n_params + len(out_names)))

    def _body(*args):
        operands = list(args)
        if part_name is not None:
            operands.append(partition_id_tensor())
        outs = _bass_exec_p.bind(
            *operands,
            out_avals=tuple(out_avals),
            in_names=tuple(all_names),
            out_names=tuple(out_names),
            lowering_input_output_aliases=(),
            sim_require_finite=True,
            sim_require_nnan=True,
            nc=nc,
        )
        return tuple(outs)

    _C["bass"] = jax.jit(
        shard_map(_body, mesh=mesh,
                  in_specs=(PS("core"),) * (n_params + len(out_names)),
                  out_specs=(PS("core"),) * len(out_names),
                  check_rep=False),
        donate_argnums=donate, keep_unused=True)
    _C["zero_shapes"] = zero_shapes
    _C["ready"] = True


def kernel(tokens, embed, w_ih_f, w_hh_f, b_f, w_ih_b, w_hh_b, b_b,
           w_enc, b_enc, w_out, b_out, trans):
    tokens = np.asarray(tokens)
    if "ready" not in _C:
        _setup(np.asarray(embed, np.float32),
               np.asarray(w_ih_f, np.float32), np.asarray(b_f, np.float32),
               np.asarray(w_ih_b, np.float32), np.asarray(b_b, np.float32),
               np.asarray(w_hh_f, np.float32), np.asarray(w_hh_b, np.float32),
               np.asarray(w_enc, np.float32), np.asarray(b_enc, np.float32),
               np.asarray(w_out, np.float32), np.asarray(b_out, np.float32),
               np.asarray(trans, np.float32))
    jax = _C["jax"]

    # tokens [T, B] -> global [8*T, CB] (core-major)
    tok_g = np.ascontiguousarray(
        tokens.reshape(T, 8, CB).transpose(1, 0, 2).reshape(8 * T, CB))
    tok_dev = jax.device_put(tok_g, _C["shard"])
    xp_dev = _C["gather"](tok_dev, _C["WE"])

    w = _C["wdevs"]
    zeros = [np.zeros((8 * s[0],) + s[1:], dt) for s, dt in _C["zero_shapes"]]
    zbuf, afin = _C["bass"](xp_dev, w["whh"], w["wenc"], w["benc"], w["wout"],
                            w["bout"], w["pp"], w["a0"], *zeros)
    zbuf = np.asarray(zbuf).astype(np.float64)    # [8, CB, NZ]
    afin = np.asarray(afin).astype(np.float64)    # [8*K, CB]

    etstop = _C["etstop"]
    out = np.empty((B,), np.float32)
    for c in range(8):
        zb = zbuf[c]                              # [CB, NZ]
        af = afin[c * K:(c + 1) * K]              # [K, CB]
        lz = np.log(zb).sum(axis=1) + np.log(etstop @ af)
        out[c * CB:(c + 1) * CB] = lz.astype(np.float32)
    return out


# revision 12
# speedup vs baseline: 2.7408x; 2.7408x over previous
"""BiLSTM-CRF forward (log partition) on 8 trn2 NeuronCores — single launch.

Architecture (per call, after one-time cached setup):
  host:  tokens [512,64] -> per-core token blocks (128KB shipped).
  jit1 (XLA, cached):  xp = WE[tokens] gather + transpose, on device.
         WE[v] = concat_d(w_ih_d @ embed[v] + b_d)  (precomputed on device
         at setup; folds the whole input-projection GEMM + embedding gather
         into one table lookup).
  jit2 (Bass, cached): per core = 8 batch columns end-to-end:
         fwd+bwd LSTM scans (interleaved per step), encoder GEMM + tanh,
         emission GEMM + exp, CRF forward scan in exp domain with periodic
         rescaling -> zbuf/afin (tiny outputs).
  host:  log_z = sum(log zbuf) + log(etstop @ afin).

All weights and the WE table live on device between calls; both jits are
compiled once and cached in-process. Per-call traffic: ~128KB in, ~20KB out.
"""
import numpy as np
import ml_dtypes

import concourse.bass as bass
import concourse.mybir as mybir
import concourse.tile as tile

T, B, E, H, V, K = 512, 64, 256, 512, 50000, 50
P = 128
CB = 8             # batch columns per core
NG = 16            # gate tiles (4H/128)
NK = 4             # h chunks (H/128)
GRP = 32           # scan steps per xp prefetch group
NGRP = T // GRP    # 16
TB2 = T * CB       # 4096
NBLK2 = TB2 // 512 # 8
NZ = T // 16       # 32 rescale slots
# wpack column layout (bf16, [P, WCOLS]); f32 biases stored rounded to bf16
OFF_WHH = 0
OFF_WENC = OFF_WHH + 2 * NG * NK * P    # 16384
OFF_WOUT = OFF_WENC + 2 * NK * NK * P   # +4096
OFF_PP = OFF_WOUT + NK * K              # +200
OFF_A0 = OFF_PP + (K + 2)               # +52
OFF_BENC = OFF_A0 + CB                  # +8
OFF_BOUT = OFF_BENC + NK                # +4
WCOLS = OFF_BOUT + 1
AF = mybir.ActivationFunctionType
BF16 = mybir.dt.bfloat16
F32 = mybir.dt.float32
BF = ml_dtypes.bfloat16

GPERM = np.concatenate([
    np.arange(0, 512), np.arange(512, 1024),
    np.arange(1536, 2048), np.arange(1024, 1536)])  # i,f,o,g tile order

_C = {}


def _fix_sync_waits(nc, max_waits=1):
    import bass_rust
    for fn in nc.m.functions:
        for bb in fn.blocks:
            out = []
            for inst in bb.instructions:
                si = inst.sync_info
                if si is not None and si.on_wait and len(si.on_wait) > max_waits:
                    waits = list(si.on_wait)
                    extra, keep = waits[:-max_waits], waits[-max_waits:]
                    for j in range(0, len(extra), max_waits):
                        nop = mybir.InstNoOp(name=f"{inst.name}_ws{j}", ins=[], outs=[])
                        nop.engine = inst.engine
                        nop.sync_info = bass_rust.SyncInfo(
                            on_wait=extra[j:j + max_waits], on_update=[])
                        out.append(nop)
                    inst.sync_info = bass_rust.SyncInfo(
                        on_wait=keep, on_update=list(si.on_update or []))
                out.append(inst)
            bb.instructions = out


def build_fused(mode="all"):
    do_scan = mode in ("all", "scan", "scanlite")
    do_tail = mode in ("all", "tail")
    lite = mode == "scanlite"
    nc = bass.Bass()
    dp = nc.declare_dram_parameter
    xp_in = dp("xp", [2, NG, P, TB2], BF16, isOutput=False)
    wpack_in = dp("wpack", [P, WCOLS], BF16, isOutput=False)
    opack_out = dp("opack", [K + 4, 64], F32, isOutput=True)

    with tile.TileContext(nc) as tc:
        with tc.tile_pool(name="hseq", bufs=1) as hp:
            hs = [hp.tile([P, NK, TB2], BF16, name=f"hseq{d}", tag=f"hseq{d}")
                  for d in range(2)]

            # ---------------- LSTM scans (fwd+bwd interleaved) -----------
            if do_scan:
              with (
                tc.tile_pool(name="scanw", bufs=1) as sw,
                tc.tile_pool(name="ps", bufs=2, space="PSUM") as psp,
              ):
                whh = sw.tile([P, 2 * NG * NK, P], BF16)
                nc.sync.dma_start(
                    whh[:].rearrange("p a b -> p (a b)"),
                    wpack_in[:, OFF_WHH:OFF_WHH + 2 * NG * NK * P])
                h0 = sw.tile([P, NK * CB], BF16)
                nc.any.memset(h0[:], 0.0)
                xr = [sw.tile([P, 2, NG, GRP, CB], BF16, name=f"xr{d}", tag=f"xr{d}")
                      for d in range(2)]
                cst, gs, sio, tg, m1, m2, tcc = [], [], [], [], [], [], []
                for d in range(2):
                    cst.append(sw.tile([P, NK * CB], F32, name=f"cst{d}", tag=f"cst{d}"))
                    nc.any.memset(cst[d][:], 0.0)
                    gs.append(sw.tile([P, NG * CB], F32, name=f"gs{d}", tag=f"gs{d}"))
                    sio.append(sw.tile([P, 3 * NK * CB], F32, name=f"sio{d}", tag=f"sio{d}"))
                    tg.append(sw.tile([P, NK * CB], F32, name=f"tg{d}", tag=f"tg{d}"))
                    m1.append(sw.tile([P, NK * CB], F32, name=f"m1{d}", tag=f"m1{d}"))
                    m2.append(sw.tile([P, NK * CB], F32, name=f"m2{d}", tag=f"m2{d}"))
                    tcc.append(sw.tile([P, NK * CB], F32, name=f"tcc{d}", tag=f"tcc{d}"))

                def prefetch(g, d):
                    if g >= NGRP:
                        return
                    blk = g if d == 0 else NGRP - 1 - g
                    for m in range(NG):
                        nc.sync.dma_start(
                            xr[d][:, g % 2, m, :, :].rearrange("p t b -> p (t b)"),
                            xp_in[d, m, :, blk * GRP * CB:(blk + 1) * GRP * CB])

                for g in range(2):
                    for d in range(2):
                        prefetch(g, d)

                for g in range(NGRP):
                    for tm in range(GRP):
                        s = g * GRP + tm
                        for d in range(2):
                            t = s if d == 0 else T - 1 - s
                            tl = tm if d == 0 else GRP - 1 - tm
                            if s == 0:
                                hin = h0[:].rearrange("p (a b) -> p a b", b=CB)
                            else:
                                tp = t - 1 if d == 0 else t + 1
                                hin = hs[d][:, :, tp * CB:(tp + 1) * CB]
                            gp = psp.tile([P, NG * CB], F32, tag=f"gp{d}")
                            for m in range(NG):
                                for k in range(NK):
                                    nc.tensor.matmul(
                                        gp[:, m * CB:(m + 1) * CB],
                                        lhsT=whh[:, d * NG * NK + m * NK + k, :],
                                        rhs=hin[:, k, :],
                                        start=(k == 0), stop=(k == NK - 1))
                            if lite:
                                nc.vector.tensor_copy(
                                    hs[d][:, :, t * CB:(t + 1) * CB],
                                    gp[:, 0:NK * CB].rearrange(
                                        "p (a b) -> p a b", b=CB))
                                continue
                            nc.vector.tensor_tensor(
                                gs[d][:].rearrange("p (m b) -> p m b", b=CB),
                                gp[:].rearrange("p (m b) -> p m b", b=CB),
                                xr[d][:, g % 2, :, tl, :], mybir.AluOpType.add)
                            nc.scalar.activation(sio[d][:], gs[d][:, 0:3 * NK * CB],
                                                 AF.Sigmoid)
                            nc.scalar.activation(tg[d][:],
                                                 gs[d][:, 3 * NK * CB:4 * NK * CB],
                                                 AF.Tanh)
                            nc.vector.tensor_mul(m1[d][:], sio[d][:, 0:NK * CB],
                                                 tg[d][:])
                            nc.vector.tensor_mul(m2[d][:],
                                                 sio[d][:, NK * CB:2 * NK * CB],
                                                 cst[d][:])
                            nc.vector.tensor_add(cst[d][:], m1[d][:], m2[d][:])
                            nc.scalar.activation(tcc[d][:], cst[d][:], AF.Tanh)
                            nc.vector.tensor_mul(
                                hs[d][:, :, t * CB:(t + 1) * CB],
                                sio[d][:, 2 * NK * CB:3 * NK * CB].rearrange(
                                    "p (a b) -> p a b", b=CB),
                                tcc[d][:].rearrange("p (a b) -> p a b", b=CB))
                    for d in range(2):
                        prefetch(g + 2, d)

            # ---------------- encoder + emissions + CRF ------------------
            if do_tail:
              with (
                tc.tile_pool(name="enc", bufs=1) as ec,
                tc.tile_pool(name="eps", bufs=2, space="PSUM") as eps,
              ):
                if not do_scan:
                    nc.any.memset(hs[0][:], 0.0)
                    nc.any.memset(hs[1][:], 0.0)
                wenc = ec.tile([P, 2 * NK * NK, P], BF16)
                nc.sync.dma_start(
                    wenc[:].rearrange("p a b -> p (a b)"),
                    wpack_in[:, OFF_WENC:OFF_WENC + 2 * NK * NK * P])
                benc_bf = ec.tile([P, NK], BF16)
                nc.sync.dma_start(benc_bf[:],
                                  wpack_in[:, OFF_BENC:OFF_BENC + NK])
                benc = ec.tile([P, NK], F32)
                nc.vector.tensor_copy(benc[:], benc_bf[:])
                wout = ec.tile([P, NK, K], BF16)
                nc.sync.dma_start(
                    wout[:].rearrange("p a b -> p (a b)"),
                    wpack_in[:, OFF_WOUT:OFF_WOUT + NK * K])
                bout_bf = ec.tile([K, 1], BF16)
                nc.sync.dma_start(bout_bf[:], wpack_in[0:K, OFF_BOUT:OFF_BOUT + 1])
                bout = ec.tile([K, 1], F32)
                nc.vector.tensor_copy(bout[:], bout_bf[:])
                states = ec.tile([P, NK, TB2], BF16)

                for blk in range(NBLK2):
                    sl = slice(blk * 512, (blk + 1) * 512)
                    for m in range(NK):
                        ps = eps.tile([P, 512], F32, tag="enc")
                        for k in range(NK):
                            nc.tensor.matmul(ps[:], lhsT=wenc[:, m * NK + k, :],
                                             rhs=hs[0][:, k, sl],
                                             start=(k == 0), stop=False)
                        for k in range(NK):
                            nc.tensor.matmul(ps[:],
                                             lhsT=wenc[:, NK * NK + m * NK + k, :],
                                             rhs=hs[1][:, k, sl], start=False,
                                             stop=(k == NK - 1))
                        nc.scalar.activation(states[:, m, sl], ps[:], AF.Tanh,
                                             bias=benc[:, m:m + 1])

                expE = ec.tile([K, TB2], F32)
                for blk in range(NBLK2):
                    sl = slice(blk * 512, (blk + 1) * 512)
                    ps = eps.tile([K, 512], F32, tag="emit")
                    for k in range(NK):
                        nc.tensor.matmul(ps[:], lhsT=wout[:, k, :],
                                         rhs=states[:, k, sl],
                                         start=(k == 0), stop=(k == NK - 1))
                    nc.scalar.activation(expE[:, sl], ps[:], AF.Exp,
                                         bias=bout[:, 0:1])

                pp = ec.tile([K, K + 2], BF16)
                nc.sync.dma_start(pp[:], wpack_in[0:K, OFF_PP:OFF_PP + K + 2])
                ones_r = ec.tile([1, K], BF16)
                nc.any.memset(ones_r[:], 1.0)
                A = ec.tile([K, CB], BF16)
                nc.sync.dma_start(A[:], wpack_in[0:K, OFF_A0:OFF_A0 + CB])
                zbuf = ec.tile([1, CB, NZ], F32)
                izb = ec.tile([1, CB], F32)
                izb_bf = ec.tile([1, CB], BF16)

                for t in range(T):
                    ps = eps.tile([K, CB], F32, tag="crf", bufs=1)
                    nc.tensor.matmul(ps[:], lhsT=pp[:, 0:K], rhs=A[:],
                                     start=True, stop=True)
                    if t % 16 == 15:
                        r = t // 16
                        zps = eps.tile([1, CB], F32, tag="zps", bufs=1)
                        nc.tensor.matmul(zps[:], lhsT=pp[:, K:K + 1], rhs=A[:],
                                         start=True, stop=True)
                        nc.vector.tensor_copy(zbuf[:, :, r], zps[:])
                        nc.vector.reciprocal(izb[:], zps[:])
                        nc.vector.tensor_copy(izb_bf[:], izb[:])
                        zb = eps.tile([K, CB], F32, tag="zbc", bufs=1)
                        nc.tensor.matmul(zb[:], lhsT=ones_r[:], rhs=izb_bf[:],
                                         start=True, stop=True)
                        nc.vector.tensor_mul(A[:], ps[:],
                                             expE[:, t * CB:(t + 1) * CB])
                        nc.vector.tensor_mul(A[:], A[:], zb[:])
                    else:
                        nc.vector.tensor_mul(A[:], ps[:],
                                             expE[:, t * CB:(t + 1) * CB])

                af = ec.tile([K, CB], F32)
                nc.vector.tensor_copy(af[:], A[:])
                nc.sync.dma_start(opack_out[0:K, 0:CB], af[:])
                nc.sync.dma_start(
                    opack_out[K:K + 4, :].rearrange("a b -> () (a b)"),
                    zbuf[:].rearrange("o a b -> o (a b)"))

    _fix_sync_waits(nc)
    return nc


def _tiles_T(W, nm, nk):
    """W [nm*128, nk*128] -> [128, nm*nk, 128] with [:, m*nk+k, :] = block(m,k).T"""
    return np.ascontiguousarray(
        W.reshape(nm, P, nk, P).transpose(3, 0, 2, 1).reshape(P, nm * nk, P))


def _rep8(a):
    """concat 8 copies along axis 0 (replicated shard_map input)."""
    return np.ascontiguousarray(
        np.broadcast_to(a, (8,) + a.shape).reshape((8 * a.shape[0],) + a.shape[1:]))


def _setup(embed, w_ih_f, b_f, w_ih_b, b_b, w_hh_f, w_hh_b,
           w_enc, b_enc, w_out, b_out, trans):
    import jax
    import jax.numpy as jnp
    from jax.experimental.shard_map import shard_map
    from jax.sharding import Mesh, PartitionSpec as PS, NamedSharding
    from concourse.bass2jax import (install_neuronx_cc_hook, _bass_exec_p,
                                    partition_id_tensor)

    install_neuronx_cc_hook()
    devs = jax.devices()[:8]
    mesh = Mesh(np.asarray(devs), ("core",))
    shard = NamedSharding(mesh, PS("core"))
    _C["mesh"] = mesh
    _C["shard"] = shard
    _C["jax"] = jax

    # ---- WE table: WE[v, d, m, p] = (w_ih_d[GPERM] @ embed[v]) + b_d[GPERM]
    wihg = np.stack([w_ih_f[GPERM], w_ih_b[GPERM]]).astype(BF)      # [2,4H,E]
    bg = np.stack([b_f[GPERM], b_b[GPERM]]).astype(np.float32)      # [2,4H]
    embed_b = embed.astype(BF)                                      # [V,E]

    def we_fn(embed_c, wihg_c, bg_c):
        out = jnp.einsum("ve,dge->vdg", embed_c, wihg_c,
                         preferred_element_type=jnp.float32)
        out = out + bg_c[None]
        return out.astype(jnp.bfloat16).reshape(V, 2, NG, P)

    we_jit = jax.jit(shard_map(we_fn, mesh=mesh, in_specs=(PS("core"),) * 3,
                               out_specs=PS("core")))
    emb_dev = jax.device_put(_rep8(embed_b), shard)
    wihg_dev = jax.device_put(_rep8(wihg), shard)
    bg_dev = jax.device_put(_rep8(bg), shard)
    WE_dev = we_jit(emb_dev, wihg_dev, bg_dev)
    WE_dev.block_until_ready()
    del emb_dev, wihg_dev, bg_dev
    _C["WE"] = WE_dev

    # ---- gather jit: tokens -> xp layout [2, NG, P, TB2] per core
    def gather_fn(tok_c, WE_c):
        xp = WE_c[tok_c.reshape(-1)]          # [TB2, 2, NG, P]
        return jnp.transpose(xp, (1, 2, 3, 0))

    _C["gather"] = jax.jit(shard_map(
        gather_fn, mesh=mesh, in_specs=(PS("core"), PS("core")),
        out_specs=PS("core")))

    # ---- device-resident packed bass weights [P, WCOLS] bf16
    whh_t = np.concatenate(
        [_tiles_T(w_hh_f[GPERM], NG, NK), _tiles_T(w_hh_b[GPERM], NG, NK)],
        axis=1)                                                     # [P,128,P]
    wenc_t = np.concatenate(
        [_tiles_T(w_enc[:, :H], NK, NK), _tiles_T(w_enc[:, H:], NK, NK)],
        axis=1)                                                     # [P,32,P]
    wout_t = np.ascontiguousarray(
        w_out.reshape(K, NK, P).transpose(2, 1, 0))                 # [P,NK,K]
    ppm = np.zeros((K, K + 2), np.float32)
    ppm[:, :K] = np.exp(trans.astype(np.float64)).T.astype(np.float32)
    ppm[:, K] = 1.0
    ppm[:, K + 1] = np.exp(trans[K - 1].astype(np.float64)).astype(np.float32)
    a0 = np.zeros((K, CB), np.float32)
    a0[0, :] = 1.0

    wpack = np.zeros((P, WCOLS), np.float32)
    wpack[:, OFF_WHH:OFF_WHH + 2 * NG * NK * P] = whh_t.reshape(P, -1)
    wpack[:, OFF_WENC:OFF_WENC + 2 * NK * NK * P] = wenc_t.reshape(P, -1)
    wpack[:, OFF_WOUT:OFF_WOUT + NK * K] = wout_t.reshape(P, -1)
    wpack[:K, OFF_PP:OFF_PP + K + 2] = ppm
    wpack[:K, OFF_A0:OFF_A0 + CB] = a0
    wpack[:, OFF_BENC:OFF_BENC + NK] = b_enc.reshape(NK, P).T
    wpack[:K, OFF_BOUT] = b_out
    _C["wpack"] = jax.device_put(_rep8(wpack.astype(BF)), shard)
    _C["etstop"] = np.exp(trans[K - 1].astype(np.float64))

    # ---- bass jit (built once, cached)
    nc = build_fused()
    part_name = nc.partition_id_tensor.name if nc.partition_id_tensor else None
    in_names, out_names, out_avals, zero_shapes = [], [], [], []
    for alloc in nc.m.functions[0].allocations:
        if not isinstance(alloc, mybir.MemoryLocationSet):
            continue
        name = alloc.memorylocations[0].name
        if alloc.kind == "ExternalInput":
            if name != part_name:
                in_names.append(name)
        elif alloc.kind == "ExternalOutput":
            out_names.append(name)
            shape = tuple(alloc.tensor_shape)
            dtype = mybir.dt.np(alloc.dtype)
            out_avals.append(jax.core.ShapedArray(shape, dtype))
            zero_shapes.append((shape, dtype))
    assert in_names == ["xp", "wpack"], in_names
    assert out_names == ["opack"], out_names
    n_params = len(in_names)
    all_names = in_names + out_names
    if part_name is not None:
        all_names = all_names + [part_name]
    donate = tuple(range(n_params, n_params + len(out_names)))

    def _body(*args):
        operands = list(args)
        if part_name is not None:
            operands.append(partition_id_tensor())
        outs = _bass_exec_p.bind(
            *operands,
            out_avals=tuple(out_avals),
            in_names=tuple(all_names),
            out_names=tuple(out_names),
            lowering_input_output_aliases=(),
            sim_require_finite=True,
            sim_require_nnan=True,
            nc=nc,
        )
        return tuple(outs)

    _C["bass"] = jax.jit(
        shard_map(_body, mesh=mesh,
                  in_specs=(PS("core"),) * (n_params + len(out_names)),
                  out_specs=(PS("core"),) * len(out_names),
                  check_rep=False),
        donate_argnums=donate, keep_unused=True)
    _C["zero_shapes"] = zero_shapes
    _C["ready"] = True


def kernel(tokens, embed, w_ih_f, w_hh_f, b_f, w_ih_b, w_hh_b, b_b,
           w_enc, b_enc, w_out, b_out, trans):
    tokens = np.asarray(tokens)
    if "ready" not in _C:
        _setup(np.asarray(embed, np.float32),
               np.asarray(w_ih_f, np.float32), np.asarray(b_f, np.float32),
               np.asarray(w_ih_b, np.float32), np.asarray(b_b, np.float32),
               np.asarray(w_hh_f, np.float32), np.asarray(w_hh_b, np.float32),
               np.asarray(w_enc, np.float32), np.asarray(b_enc, np.float32),
               np.asarray(w_out, np.float32), np.asarray(b_out, np.float32),
               np.asarray(trans, np.float32))
    jax = _C["jax"]

    # tokens [T, B] -> global [8*T, CB] (core-major)
    tok_g = np.ascontiguousarray(
        tokens.reshape(T, 8, CB).transpose(1, 0, 2).reshape(8 * T, CB))
    tok_dev = jax.device_put(tok_g, _C["shard"])
    xp_dev = _C["gather"](tok_dev, _C["WE"])

    zeros = [np.zeros((8 * s[0],) + s[1:], dt) for s, dt in _C["zero_shapes"]]
    (opack,) = _C["bass"](xp_dev, _C["wpack"], *zeros)
    opack = np.asarray(opack).astype(np.float64).reshape(8, K + 4, 64)

    etstop = _C["etstop"]
    out = np.empty((B,), np.float32)
    for c in range(8):
        af = opack[c, :K, :CB]                    # [K, CB]
        zb = opack[c, K:K + 4, :].reshape(CB, NZ)  # [CB, NZ]
        lz = np.log(zb).sum(axis=1) + np.log(etstop @ af)
        out[c * CB:(c + 1) * CB] = lz.astype(np.float32)
    return out
